# revision 7
# baseline (speedup 1.0000x reference)
"""Trainium2 Bass kernel for nn_MatSurfGcn (GCN message passing, memory-bound).

Strategy (column-parallel over W_g1's output dim, 8 cores):
  reference =  enc -> gcn_conv(W_g1) -> gcn_conv(W_g2) -> head
  Using A @ (X @ W) == (A @ X) @ W and linearity of both convs:
    x0  = relu(encoders)            [14, 4096]   (tiny, replicated on-device)
    x0' = A @ x0                    [14, 4096]   (pre-aggregation, on-device)
    x1  = x0' @ W_g1 + b_g1         [14, 8192]   (the big memory-bound matmul,
                                                  column-sharded 1024/core)
    t   = x1 @ W_g2                 [14, 1]      (row-sharded partial per core)
    host: y = W_head . (A @ sum_c t_c + b_g2) + b_head
  Each core streams its 16.8 MB shard of W_g1 from HBM exactly once
  (the roofline) while the PE consumes it as stationary matmul weights.
"""

import os

import numpy as np

D1, D2 = 4096, 8192
N = 14
NCORES = 8
SH = D2 // NCORES        # 1024 W_g1 columns per core
MB = SH // 128           # 8 column blocks of 128 per core
KC = D1 // 128           # 32 contraction chunks of 128
CPT = int(os.environ.get("KERNEL_CPT", "2"))  # k-chunks per DMA (1 MiB default)
WBUFS = int(os.environ.get("KERNEL_WBUFS", "4"))  # wg1 tile double-buffers
ENC_K = 18               # 6+1 mats, 3+1 cyls, 4+1 planes, 1+1 power rows

_CACHE = {}


def _build_nc():
    import concourse.bacc as bacc
    import concourse.bass as bass
    import concourse.mybir as mybir
    import concourse.tile as tile

    f32 = mybir.dt.float32
    relu = mybir.ActivationFunctionType.Relu
    psum = bass.MemorySpace.PSUM

    nc = bacc.Bacc(
        "TRN2", target_bir_lowering=False, debug=False, enable_asserts=False
    )

    wenc_d = nc.dram_tensor("wenc", [ENC_K, D1], f32, kind="ExternalInput")
    s_d = nc.dram_tensor("s", [ENC_K, N], f32, kind="ExternalInput")
    agg_d = nc.dram_tensor("agg", [N, N], f32, kind="ExternalInput")
    # host-swizzled: [(kt p), (a n)] with kt = k-pair index, a = k within pair
    wg1_d = nc.dram_tensor(
        "wg1", [(KC // CPT) * 128, CPT * SH], f32, kind="ExternalInput"
    )
    brep_d = nc.dram_tensor("brep", [128, MB * N], f32, kind="ExternalInput")
    w2_d = nc.dram_tensor("w2", [128, MB], f32, kind="ExternalInput")
    t_d = nc.dram_tensor("t", [N, 1], f32, kind="ExternalOutput")

    with tile.TileContext(nc) as tc:
        with (
            tc.tile_pool(name="const", bufs=1) as cpool,
            tc.tile_pool(name="wg1p", bufs=WBUFS) as wpool,
            tc.tile_pool(name="encps", bufs=2, space=psum) as eps,
            tc.tile_pool(name="encps2", bufs=2, space=psum) as eps2,
            tc.tile_pool(name="zps", bufs=1, space=psum) as zps,
            tc.tile_pool(name="tps", bufs=1, space=psum) as tps,
            tc.tile_pool(name="work", bufs=3) as sbp,
        ):
            wenc_sb = cpool.tile([ENC_K, D1], f32)
            nc.sync.dma_start(out=wenc_sb[:], in_=wenc_d[:])
            s_sb = cpool.tile([ENC_K, N], f32)
            nc.sync.dma_start(out=s_sb[:], in_=s_d[:])
            agg_sb = cpool.tile([N, N], f32)
            nc.sync.dma_start(out=agg_sb[:], in_=agg_d[:])
            brep_sb = cpool.tile([128, MB * N], f32)
            nc.sync.dma_start(out=brep_sb[:], in_=brep_d[:])
            w2_sb = cpool.tile([128, MB], f32)
            nc.sync.dma_start(out=w2_sb[:], in_=w2_d[:])

            # x0'T = (A @ relu(S.T @ Wenc)).T, stored chunk-by-chunk:
            # column block k holds chunk k of D1 -> [128, 32*14]
            x0pt = cpool.tile([128, KC * N], f32)
            for k in range(KC):
                pa = eps.tile([N, 128], f32)
                nc.tensor.matmul(
                    pa[:],
                    s_sb[:],
                    wenc_sb[:, k * 128 : (k + 1) * 128],
                    start=True,
                    stop=True,
                )
                x0k = sbp.tile([N, 128], f32)
                nc.scalar.activation(x0k[:], pa[:], relu)
                pb = eps2.tile([128, N], f32)
                nc.tensor.matmul(pb[:], x0k[:], agg_sb[:], start=True, stop=True)
                nc.vector.tensor_copy(x0pt[:, k * N : (k + 1) * N], pb[:])

            # zT[m-block] [128, 14] = sum_k W_g1[k,m].T @ x0'T[k]; all 8
            # m-blocks live in ONE psum bank (8*14*4B = 448B). Exactly one
            # start (clears the bank's has_written bits) and one stop.
            zt = zps.tile([128, MB * N], f32)
            for kt in range(KC // CPT):
                wt = wpool.tile([128, CPT * SH], f32, tag="wt")
                nc.sync.dma_start(
                    out=wt[:], in_=wg1_d[kt * 128 : (kt + 1) * 128, :]
                )
                for a in range(CPT):
                    k = kt * CPT + a
                    for m in range(MB):
                        nc.tensor.matmul(
                            zt[:, m * N : (m + 1) * N],
                            wt[:, a * SH + m * 128 : a * SH + (m + 1) * 128],
                            x0pt[:, k * N : (k + 1) * N],
                            start=(k == 0 and m == 0),
                            stop=(k == KC - 1 and m == MB - 1),
                        )

            # x1T = zT + b_g1 (pre-replicated host-side to match layout)
            x1t = sbp.tile([128, MB * N], f32)
            nc.vector.tensor_add(x1t[:], zt[:], brep_sb[:])

            # t = x1 @ w2 shard: accumulate the 8 column blocks
            tp = tps.tile([N, 1], f32)
            for m in range(MB):
                nc.tensor.matmul(
                    tp[:],
                    x1t[:, m * N : (m + 1) * N],
                    w2_sb[:, m : m + 1],
                    start=(m == 0),
                    stop=(m == MB - 1),
                )
            t_sb = sbp.tile([N, 1], f32)
            nc.vector.tensor_copy(t_sb[:], tp[:])
            nc.sync.dma_start(out=t_d[:], in_=t_sb[:])

    nc.compile()
    return nc


def get_nc():
    if "nc" not in _CACHE:
        _CACHE["nc"] = _build_nc()
    return _CACHE["nc"]


def build_graph_matrix(edge_index):
    """Dense normalized adjacency of the PyG-style GCNConv (self-loops +
    symmetric deg^{-1/2}); multi-edges accumulate like segment_sum does."""
    ei = np.concatenate(
        [edge_index.astype(np.int64), np.stack([np.arange(N), np.arange(N)])],
        axis=1,
    )
    src, dst = ei[0], ei[1]
    deg = np.zeros(N, np.float32)
    np.add.at(deg, dst, np.ones(len(dst), np.float32))
    dis = np.where(deg > 0, 1.0 / np.sqrt(np.maximum(deg, 1e-12)), 0.0).astype(
        np.float32
    )
    A = np.zeros((N, N), np.float32)
    np.add.at(A, (dst, src), dis[src] * dis[dst])
    return A


def build_host_inputs(inputs):
    """Per-core input maps + the graph matrix for the host epilogue."""
    f32 = np.float32
    mats = np.asarray(inputs["mats"], f32)
    cyls = np.asarray(inputs["cyls"], f32)
    planes = np.asarray(inputs["planes"], f32)
    power = np.asarray(inputs["power"], f32)
    edge_index = np.asarray(inputs["edge_index"])

    A = build_graph_matrix(edge_index)

    # Block-diagonal node features with bias rows of ones: x0 = relu(S.T @ Wenc)
    S = np.zeros((ENC_K, N), f32)
    S[0:6, 0:6] = mats.T
    S[6, 0:6] = 1.0
    S[7:10, 6:10] = cyls.T
    S[10, 6:10] = 1.0
    S[11:15, 10:13] = planes.T
    S[15, 10:13] = 1.0
    S[16, 13] = power[0] / 10000.0
    S[17, 13] = 1.0

    Wenc = np.ascontiguousarray(
        np.concatenate(
            [
                np.asarray(inputs["W_mat"], f32),
                np.asarray(inputs["b_mat"], f32)[None, :],
                np.asarray(inputs["W_cyl"], f32),
                np.asarray(inputs["b_cyl"], f32)[None, :],
                np.asarray(inputs["W_pl"], f32),
                np.asarray(inputs["b_pl"], f32)[None, :],
                np.asarray(inputs["W_pw"], f32),
                np.asarray(inputs["b_pw"], f32)[None, :],
            ],
            axis=0,
        )
    )
    assert Wenc.shape == (ENC_K, D1)

    W_g1 = np.asarray(inputs["W_g1"], f32)
    b_g1 = np.asarray(inputs["b_g1"], f32)
    W_g2 = np.asarray(inputs["W_g2"], f32)

    AGG = np.ascontiguousarray(A.T)  # device computes x0T_k @ A.T

    in_maps = []
    for c in range(NCORES):
        sl = slice(c * SH, (c + 1) * SH)
        wg1_c = np.ascontiguousarray(
            W_g1[:, sl]
            .reshape(KC // CPT, CPT, 128, SH)
            .transpose(0, 2, 1, 3)
            .reshape((KC // CPT) * 128, CPT * SH)
        )
        b_sh = b_g1[sl].reshape(MB, 128)
        brep_c = np.ascontiguousarray(
            np.repeat(b_sh.T[:, :, None], N, axis=2).reshape(128, MB * N)
        )
        w2_c = np.ascontiguousarray(W_g2[sl, 0].reshape(MB, 128).T)
        in_maps.append(
            {
                "wenc": Wenc,
                "s": S,
                "agg": AGG,
                "wg1": wg1_c,
                "brep": brep_c,
                "w2": w2_c,
            }
        )
    return in_maps, A


def epilogue(t_parts, A, inputs):
    f32 = np.float32
    b_g2 = np.asarray(inputs["b_g2"], f32)
    W_head = np.asarray(inputs["W_head"], f32)
    b_head = np.asarray(inputs["b_head"], f32)
    t = np.add.reduce([p.astype(f32) for p in t_parts])  # [14, 1]
    x2 = A @ t + b_g2[0]  # [14, 1]
    y = float(x2[:, 0] @ W_head[:, 0]) + float(b_head[0])
    return np.array([y], dtype=f32)


def run_on_hw(in_maps, trace=False, tmpdir=None):
    from concourse.bass_utils import run_bass_kernel_spmd

    nc = get_nc()
    return run_bass_kernel_spmd(
        nc,
        in_maps,
        core_ids=list(range(NCORES)),
        trace=trace,
        tmpdir=tmpdir,
    )


def kernel(**inputs):
    in_maps, A = build_host_inputs(inputs)
    res = run_on_hw(in_maps, trace=bool(int(os.environ.get("KERNEL_TRACE", "0"))))
    _CACHE["last_result"] = res
    t_parts = [r["t"] for r in res.results]
    return epilogue(t_parts, A, inputs)


# revision 8
# speedup vs baseline: 1.0234x; 1.0234x over previous
"""Trainium2 Bass kernel for nn_MatSurfGcn (GCN message passing, memory-bound).

Strategy (column-parallel over W_g1's output dim, 8 cores):
  reference =  enc -> gcn_conv(W_g1) -> gcn_conv(W_g2) -> head
  Using A @ (X @ W) == (A @ X) @ W and linearity of both convs:
    x0  = relu(encoders)            [14, 4096]   (tiny, on-device)
    x0' = A @ x0                    [14, 4096]   (pre-aggregation, on-device)
    z   = x0' @ W_g1                [14, 8192]   (big memory-bound matmul,
                                                  column-sharded 1024/core;
                                                  W_g1 streams as the MOVING
                                                  operand: 64 logical matmuls
                                                  of N=512, not 256 tiny ones)
    t_c = z_c @ w2_c                [14, 1]      (fused DVE mul+reduce)
    host: t = sum_c t_c + b_g1.W_g2 ; y = W_head.(A@t + b_g2) + b_head
  Each core streams its 16.8 MB shard of W_g1 from HBM exactly once
  (the memory roofline) and through the PE twice (fp32 LOW/HIGH passes).
"""

import os

import numpy as np

D1, D2 = 4096, 8192
N = 14
NCORES = 8
SH = D2 // NCORES        # 1024 W_g1 columns per core
KC = D1 // 128           # 32 contraction chunks of 128
CPT = int(os.environ.get("KERNEL_CPT", "2"))  # k-chunks per DMA (1 MiB default)
WBUFS = int(os.environ.get("KERNEL_WBUFS", "4"))  # wg1 tile double-buffers
ENC_K = 18               # 6+1 mats, 3+1 cyls, 4+1 planes, 1+1 power rows

_CACHE = {}


def _build_nc():
    import concourse.bacc as bacc
    import concourse.bass as bass
    import concourse.mybir as mybir
    import concourse.tile as tile

    f32 = mybir.dt.float32
    relu = mybir.ActivationFunctionType.Relu
    psum = bass.MemorySpace.PSUM
    alu = mybir.AluOpType

    nc = bacc.Bacc(
        "TRN2", target_bir_lowering=False, debug=False, enable_asserts=False
    )

    wenc_d = nc.dram_tensor("wenc", [ENC_K, D1], f32, kind="ExternalInput")
    s_d = nc.dram_tensor("s", [ENC_K, N], f32, kind="ExternalInput")
    agg_d = nc.dram_tensor("agg", [N, N], f32, kind="ExternalInput")
    # host-swizzled: [(kt p), (a n)] with kt = k-group index, a = k in group
    wg1_d = nc.dram_tensor(
        "wg1", [(KC // CPT) * 128, CPT * SH], f32, kind="ExternalInput"
    )
    w2b_d = nc.dram_tensor("w2b", [N, SH], f32, kind="ExternalInput")
    t_d = nc.dram_tensor("t", [N, 1], f32, kind="ExternalOutput")

    NT = SH // 512  # PSUM-bank-sized column tiles per core

    with tile.TileContext(nc) as tc:
        with (
            tc.tile_pool(name="const", bufs=1) as cpool,
            tc.tile_pool(name="wg1p", bufs=WBUFS) as wpool,
            tc.tile_pool(name="encps", bufs=2, space=psum) as eps,
            tc.tile_pool(name="encps2", bufs=2, space=psum) as eps2,
            tc.tile_pool(name="zps", bufs=1, space=psum) as zps,
            tc.tile_pool(name="work", bufs=3) as sbp,
        ):
            wenc_sb = cpool.tile([ENC_K, D1], f32)
            nc.sync.dma_start(out=wenc_sb[:], in_=wenc_d[:])
            s_sb = cpool.tile([ENC_K, N], f32)
            nc.sync.dma_start(out=s_sb[:], in_=s_d[:])
            agg_sb = cpool.tile([N, N], f32)
            nc.sync.dma_start(out=agg_sb[:], in_=agg_d[:])
            w2b_sb = cpool.tile([N, SH], f32)
            nc.sync.dma_start(out=w2b_sb[:], in_=w2b_d[:])

            # x0'T = (A @ relu(S.T @ Wenc)).T, chunk k of D1 -> [128, 32*14]
            x0pt = cpool.tile([128, KC * N], f32)
            for k in range(KC):
                pa = eps.tile([N, 128], f32)
                nc.tensor.matmul(
                    pa[:],
                    s_sb[:],
                    wenc_sb[:, k * 128 : (k + 1) * 128],
                    start=True,
                    stop=True,
                )
                x0k = sbp.tile([N, 128], f32)
                nc.scalar.activation(x0k[:], pa[:], relu)
                pb = eps2.tile([128, N], f32)
                nc.tensor.matmul(pb[:], x0k[:], agg_sb[:], start=True, stop=True)
                nc.vector.tensor_copy(x0pt[:, k * N : (k + 1) * N], pb[:])

            # z = x0' @ wg1_shard, accumulated over 32 k-chunks into
            # [14, 1024] psum (one accumulation group per 512-col bank).
            # x0'T chunk is the tiny stationary operand; wg1 streams.
            z_ps = zps.tile([N, SH], f32)
            for kt in range(KC // CPT):
                wt = wpool.tile([128, CPT * SH], f32, tag="wt")
                nc.sync.dma_start(
                    out=wt[:], in_=wg1_d[kt * 128 : (kt + 1) * 128, :]
                )
                for a in range(CPT):
                    k = kt * CPT + a
                    for nt in range(NT):
                        nc.tensor.matmul(
                            z_ps[:, nt * 512 : (nt + 1) * 512],
                            x0pt[:, k * N : (k + 1) * N],
                            wt[:, a * SH + nt * 512 : a * SH + (nt + 1) * 512],
                            start=(k == 0),
                            stop=(k == KC - 1),
                        )

            # t = z @ w2 shard as a fused row-wise mul+reduce on DVE
            prod = sbp.tile([N, SH], f32)
            t_sb = sbp.tile([N, 1], f32)
            nc.vector.tensor_tensor_reduce(
                out=prod[:],
                in0=z_ps[:],
                in1=w2b_sb[:],
                scale=1.0,
                scalar=0.0,
                op0=alu.mult,
                op1=alu.add,
                accum_out=t_sb[:],
            )
            nc.sync.dma_start(out=t_d[:], in_=t_sb[:])

    nc.compile()
    return nc


def get_nc():
    if "nc" not in _CACHE:
        _CACHE["nc"] = _build_nc()
    return _CACHE["nc"]


def build_graph_matrix(edge_index):
    """Dense normalized adjacency of the PyG-style GCNConv (self-loops +
    symmetric deg^{-1/2}); multi-edges accumulate like segment_sum does."""
    ei = np.concatenate(
        [edge_index.astype(np.int64), np.stack([np.arange(N), np.arange(N)])],
        axis=1,
    )
    src, dst = ei[0], ei[1]
    deg = np.zeros(N, np.float32)
    np.add.at(deg, dst, np.ones(len(dst), np.float32))
    dis = np.where(deg > 0, 1.0 / np.sqrt(np.maximum(deg, 1e-12)), 0.0).astype(
        np.float32
    )
    A = np.zeros((N, N), np.float32)
    np.add.at(A, (dst, src), dis[src] * dis[dst])
    return A


def build_host_inputs(inputs):
    """Per-core input maps + the graph matrix for the host epilogue."""
    f32 = np.float32
    mats = np.asarray(inputs["mats"], f32)
    cyls = np.asarray(inputs["cyls"], f32)
    planes = np.asarray(inputs["planes"], f32)
    power = np.asarray(inputs["power"], f32)
    edge_index = np.asarray(inputs["edge_index"])

    A = build_graph_matrix(edge_index)

    # Block-diagonal node features with bias rows of ones: x0 = relu(S.T @ Wenc)
    S = np.zeros((ENC_K, N), f32)
    S[0:6, 0:6] = mats.T
    S[6, 0:6] = 1.0
    S[7:10, 6:10] = cyls.T
    S[10, 6:10] = 1.0
    S[11:15, 10:13] = planes.T
    S[15, 10:13] = 1.0
    S[16, 13] = power[0] / 10000.0
    S[17, 13] = 1.0

    Wenc = np.ascontiguousarray(
        np.concatenate(
            [
                np.asarray(inputs["W_mat"], f32),
                np.asarray(inputs["b_mat"], f32)[None, :],
                np.asarray(inputs["W_cyl"], f32),
                np.asarray(inputs["b_cyl"], f32)[None, :],
                np.asarray(inputs["W_pl"], f32),
                np.asarray(inputs["b_pl"], f32)[None, :],
                np.asarray(inputs["W_pw"], f32),
                np.asarray(inputs["b_pw"], f32)[None, :],
            ],
            axis=0,
        )
    )
    assert Wenc.shape == (ENC_K, D1)

    W_g1 = np.asarray(inputs["W_g1"], f32)
    W_g2 = np.asarray(inputs["W_g2"], f32)

    AGG = np.ascontiguousarray(A.T)  # device computes x0T_k @ A.T

    in_maps = []
    for c in range(NCORES):
        sl = slice(c * SH, (c + 1) * SH)
        wg1_c = np.ascontiguousarray(
            W_g1[:, sl]
            .reshape(KC // CPT, CPT, 128, SH)
            .transpose(0, 2, 1, 3)
            .reshape((KC // CPT) * 128, CPT * SH)
        )
        w2b_c = np.ascontiguousarray(np.tile(W_g2[sl, 0][None, :], (N, 1)))
        in_maps.append(
            {"wenc": Wenc, "s": S, "agg": AGG, "wg1": wg1_c, "w2b": w2b_c}
        )
    return in_maps, A


def epilogue(t_parts, A, inputs):
    f32 = np.float32
    b_g1 = np.asarray(inputs["b_g1"], f32)
    W_g2 = np.asarray(inputs["W_g2"], f32)
    b_g2 = np.asarray(inputs["b_g2"], f32)
    W_head = np.asarray(inputs["W_head"], f32)
    b_head = np.asarray(inputs["b_head"], f32)
    t = np.add.reduce([p.astype(f32) for p in t_parts])  # [14, 1]
    t = t + np.float32(b_g1 @ W_g2[:, 0])  # bias term of conv2's input
    x2 = A @ t + b_g2[0]  # [14, 1]
    y = float(x2[:, 0] @ W_head[:, 0]) + float(b_head[0])
    return np.array([y], dtype=f32)


def run_on_hw(in_maps, trace=False, tmpdir=None):
    from concourse.bass_utils import run_bass_kernel_spmd

    nc = get_nc()
    return run_bass_kernel_spmd(
        nc,
        in_maps,
        core_ids=list(range(NCORES)),
        trace=trace,
        tmpdir=tmpdir,
    )


def kernel(**inputs):
    in_maps, A = build_host_inputs(inputs)
    res = run_on_hw(in_maps, trace=bool(int(os.environ.get("KERNEL_TRACE", "0"))))
    _CACHE["last_result"] = res
    t_parts = [r["t"] for r in res.results]
    return epilogue(t_parts, A, inputs)
